# revision 1
# baseline (speedup 1.0000x reference)
"""Trainium2 Bass kernel for nn_Confidence_Score (gnn_message_passing).

Math: with S_g = sum of x over nodes of graph g and n_g = node count,
every node of graph g has identical activations:
    h1_g = relu(S_g @ W1 + b1)
    h2_g = relu((n_g * h1_g) @ W2 + b2)
    c_g  = h2_g @ Wc + bc ;  out_node = sp/(1+sp), sp = softplus(c_g)
Kernel: (pass 1) segment-sum x -> S via one-hot matmuls, while also
building the transposed one-hot A_T[g, node] in bulk (ones-broadcast
matmul + is_eq); (MLP) tiny per-graph network; (pass 2) out = og.T @ A_T
as 512-wide matmuls with og split into exact bf16 hi+lo parts.

PE runs bf16 everywhere data allows: one-hots are exact in bf16, and
x is split x = x_hi + x_lo (both bf16, exact to ~2^-18 rel) so one
N=256 moving matmul per 128-node chunk accumulates S_hi|S_lo in PSUM.

Sharding: graph-aligned contiguous node ranges, balanced by node count,
one range per core (8 cores); weights replicated; no collectives.
"""

import os
import sys

for _p in ("/root/.axon_site", "/root/.axon_site/_ro/trn_rl_repo",
           "/root/.axon_site/_ro/pypackages", "/opt/trn_rl_repo"):
    if os.path.isdir(_p) and _p not in sys.path:
        sys.path.append(_p)

import numpy as np

N_CORES = 8
D = 128
H = 256
G_TOTAL = 512
G_PAD = 72        # max local graphs per core (actual ~66)
CHUNK = 128       # nodes per aggregation matmul
XB = 10           # chunks per x DMA
ROW = 2 * D + 2   # x_hi | x_lo | bt | pad, bf16
OB = 512          # nodes per expansion matmul / A_T-gen block

# packed f32 const layout (columns in "cpk" [128, CPK]):
C_ID = 0            # ident [128,128]
C_W1 = 128          # w1 [128,256]
C_W2A = 384         # w2 rows 0-127 [128,256]
C_W2B = 640         # w2 rows 128-255 [128,256]
C_WC = 896          # wc as 2 cols: [0:128] and [128:256]
C_NC = 898          # ncol [G_PAD,1]
C_BC = 899          # bccol [G_PAD,1]
C_IO = 900          # iota column 0..127 [128,1]
C_B1 = 901          # b1 broadcast [G_PAD,256]
C_B2 = 1157         # b2 broadcast [G_PAD,256]
CPK = 1413

_CACHE = {}


def _build(nodes_pad):
    """Build + compile the single-core Bass program (shapes uniform across cores)."""
    from contextlib import ExitStack

    import concourse.bacc as bacc
    import concourse.mybir as mybir
    import concourse.tile as tile

    f32 = mybir.dt.float32
    bf16 = mybir.dt.bfloat16
    AF = mybir.ActivationFunctionType
    OP = mybir.AluOpType

    n_chunks = nodes_pad // CHUNK
    assert n_chunks % XB == 0
    n_ob = nodes_pad // OB

    nc = bacc.Bacc("TRN2", target_bir_lowering=False, debug=False)

    xb_d = nc.dram_tensor("xb", [nodes_pad, ROW], bf16, kind="ExternalInput").ap()
    bta_d = nc.dram_tensor("btall", [1, nodes_pad], bf16, kind="ExternalInput").ap()
    cpk_d = nc.dram_tensor("cpk", [128, CPK], f32, kind="ExternalInput").ap()
    io_d = nc.dram_tensor("iotab", [128, G_PAD], bf16, kind="ExternalInput").ap()
    out_d = nc.dram_tensor("out", [n_ob, OB], f32, kind="ExternalOutput").ap()

    # host pre-shuffles xb so each (group, partition) segment is contiguous
    xb_groups = xb_d.rearrange("(g p j) d -> g p (j d)", p=CHUNK, j=XB)

    with tile.TileContext(nc) as tc, ExitStack() as ctx:
        const = ctx.enter_context(tc.tile_pool(name="const", bufs=1))
        store = ctx.enter_context(tc.tile_pool(name="store", bufs=1))
        ps_s = ctx.enter_context(tc.tile_pool(name="ps_s", bufs=1, space="PSUM"))

        cpk = const.tile([128, CPK], f32)
        nc.scalar.dma_start(cpk[:], cpk_d[:])
        ident = cpk[:, C_ID:C_ID + 128]
        w1_s = cpk[:, C_W1:C_W1 + H]
        w2a = cpk[:, C_W2A:C_W2A + H]
        w2b = cpk[:, C_W2B:C_W2B + H]
        wca = cpk[:, C_WC:C_WC + 1]
        wcb = cpk[:, C_WC + 1:C_WC + 2]
        ncs = cpk[0:G_PAD, C_NC:C_NC + 1]
        bcs = cpk[0:G_PAD, C_BC:C_BC + 1]
        io72 = cpk[0:G_PAD, C_IO:C_IO + 1]
        b1s = cpk[0:G_PAD, C_B1:C_B1 + H]
        b2s = cpk[0:G_PAD, C_B2:C_B2 + H]

        iota_b = const.tile([128, G_PAD], bf16)
        nc.scalar.dma_start(iota_b[:], io_d[:])
        btb = store.tile([G_PAD, nodes_pad], bf16)

        at_sb = store.tile([G_PAD, nodes_pad], bf16)
        es2 = store.tile([96, ((n_ob + 2) // 3) * OB], f32)
        s_ps = ps_s.tile([G_PAD, 2 * D], f32)

        # ---- pass 1: segment-sum (hi|lo bf16) + A_T generation ----
        TBW = 512
        n_tb = nodes_pad // TBW
        n_tb_per_g = -(-n_tb // (n_chunks // XB - 2))
        with (
            tc.tile_pool(name="xp", bufs=8) as xpool,
            tc.tile_pool(name="ap", bufs=8) as apool,
        ):
            for g in range(n_chunks // XB):
                xt = xpool.tile([CHUNK, XB * ROW], bf16)
                eng = nc.sync if g % 2 == 0 else nc.scalar
                eng.dma_start(xt[:], xb_groups[g])
                half = nodes_pad // 2
                if g == 0:
                    nc.scalar.dma_start(
                        btb[:, half:],
                        bta_d[0:1, half:].to_broadcast((G_PAD, nodes_pad - half)))
                if g == 1:
                    nc.sync.dma_start(
                        btb[:, 0:half],
                        bta_d[0:1, 0:half].to_broadcast((G_PAD, half)))
                for j in range(XB):
                    c = g * XB + j
                    a = apool.tile([CHUNK, G_PAD], bf16)
                    # [zero, bf16(bt)] pair bitcast to one f32 == f32(bt) exactly
                    nc.vector.tensor_scalar(
                        a[:], iota_b[:],
                        xt[:, j * ROW + 2 * D:j * ROW + 2 * D + 2].bitcast(f32),
                        None, op0=OP.is_equal,
                    )
                    nc.tensor.matmul(
                        s_ps[:], lhsT=a[:], rhs=xt[:, j * ROW:j * ROW + 2 * D],
                        start=(c == 0), stop=(c == n_chunks - 1),
                    )
                # A_T blocks, interleaved so they hide under the DMA window
                if g >= 2:
                    lo = (g - 2) * n_tb_per_g
                    hi = n_tb if g == n_chunks // XB - 1 else lo + n_tb_per_g
                    for tb in range(lo, min(hi, n_tb)):
                        nc.vector.tensor_scalar(
                            at_sb[:, tb * TBW:(tb + 1) * TBW],
                            btb[:, tb * TBW:(tb + 1) * TBW], io72[:],
                            None, op0=OP.is_equal,
                        )

        # ---- per-graph MLP ----
        with (
            tc.tile_pool(name="mlp", bufs=1) as mlp,
            tc.tile_pool(name="ps_m", bufs=2, space="PSUM") as ps_m,
        ):
            s_lo = mlp.tile([G_PAD, D], f32)
            nc.vector.tensor_copy(s_lo[:], s_ps[:, D:2 * D])
            s_sb = mlp.tile([G_PAD, D], f32)
            nc.vector.tensor_tensor(s_sb[:], s_ps[:, 0:D], s_lo[:], op=OP.add)
            st_ps = ps_m.tile([D, G_PAD], f32, tag="tps")
            nc.tensor.transpose(st_ps[:], s_sb[:], ident[0:G_PAD, 0:G_PAD])
            st_sb = mlp.tile([D, G_PAD], f32)
            nc.vector.tensor_copy(st_sb[:], st_ps[:])

            h1_ps = ps_m.tile([G_PAD, H], f32, tag="mm")
            nc.tensor.matmul(h1_ps[:], lhsT=st_sb[:], rhs=w1_s[:], start=True, stop=True)
            h1 = mlp.tile([G_PAD, H], f32)
            nc.vector.tensor_tensor(h1[:], h1_ps[:], b1s[:], op=OP.add)
            nc.vector.tensor_scalar_max(h1[:], h1[:], 0.0)
            nc.vector.tensor_scalar_mul(h1[:], h1[:], ncs[:])

            h2_ps = ps_m.tile([G_PAD, H], f32, tag="mm")
            for kk in range(2):
                tp = ps_m.tile([128, G_PAD], f32, tag="tps")
                nc.tensor.transpose(
                    tp[:], h1[:, kk * 128:(kk + 1) * 128], ident[0:G_PAD, 0:G_PAD]
                )
                tsb = mlp.tile([128, G_PAD], f32, tag=f"tsb{kk}")
                nc.vector.tensor_copy(tsb[:], tp[:])
                nc.tensor.matmul(
                    h2_ps[:], lhsT=tsb[:], rhs=(w2a[:] if kk == 0 else w2b[:]),
                    start=(kk == 0), stop=(kk == 1),
                )
            h2 = mlp.tile([G_PAD, H], f32)
            nc.vector.tensor_tensor(h2[:], h2_ps[:], b2s[:], op=OP.add)
            nc.vector.tensor_scalar_max(h2[:], h2[:], 0.0)

            c_ps = ps_m.tile([G_PAD, 1], f32, tag="mm")
            for kk in range(2):
                tp = ps_m.tile([128, G_PAD], f32, tag="tps")
                nc.tensor.transpose(
                    tp[:], h2[:, kk * 128:(kk + 1) * 128], ident[0:G_PAD, 0:G_PAD]
                )
                tsb = mlp.tile([128, G_PAD], f32, tag=f"usb{kk}")
                nc.vector.tensor_copy(tsb[:], tp[:])
                nc.tensor.matmul(
                    c_ps[:], lhsT=tsb[:], rhs=(wca[:] if kk == 0 else wcb[:]),
                    start=(kk == 0), stop=(kk == 1),
                )

            # sp = softplus(c+bc) = relu(c) + log1p(exp(-|c|)); out = sp/(1+sp)
            cc = mlp.tile([G_PAD, 1], f32)
            nc.vector.tensor_scalar_add(cc[:], c_ps[:], bcs[:])
            negc = mlp.tile([G_PAD, 1], f32)
            nc.vector.tensor_scalar_mul(negc[:], cc[:], -1.0)
            nab = mlp.tile([G_PAD, 1], f32)
            nc.vector.tensor_tensor(nab[:], cc[:], negc[:], op=OP.min)
            ex = mlp.tile([G_PAD, 1], f32)
            nc.scalar.activation(ex[:], nab[:], AF.Exp)
            ex1 = mlp.tile([G_PAD, 1], f32)
            nc.vector.tensor_scalar_add(ex1[:], ex[:], 1.0)
            lg = mlp.tile([G_PAD, 1], f32)
            nc.scalar.activation(lg[:], ex1[:], AF.Ln)
            rl = mlp.tile([G_PAD, 1], f32)
            nc.vector.tensor_scalar_max(rl[:], cc[:], 0.0)
            sp = mlp.tile([G_PAD, 1], f32)
            nc.vector.tensor_tensor(sp[:], rl[:], lg[:], op=OP.add)
            t1 = mlp.tile([G_PAD, 1], f32)
            nc.vector.tensor_scalar_add(t1[:], sp[:], 1.0)
            rcp = mlp.tile([G_PAD, 1], f32)
            nc.vector.reciprocal(rcp[:], t1[:])
            og = mlp.tile([G_PAD, 1], f32)
            nc.vector.tensor_scalar(
                og[:], rcp[:], -1.0, 1.0, op0=OP.mult, op1=OP.add
            )
            # exact bf16 hi/lo split of og
            ogh = mlp.tile([G_PAD, 1], bf16)
            nc.vector.tensor_copy(ogh[:], og[:])
            oghf = mlp.tile([G_PAD, 1], f32)
            nc.vector.tensor_copy(oghf[:], ogh[:])
            oglf = mlp.tile([G_PAD, 1], f32)
            nc.vector.tensor_tensor(oglf[:], og[:], oghf[:], op=OP.subtract)
            ogl = mlp.tile([G_PAD, 1], bf16)
            nc.vector.tensor_copy(ogl[:], oglf[:])
            zz = mlp.tile([G_PAD, 32], f32)
            nc.vector.memset(zz[:], 0.0)
            ogrh = const.tile([G_PAD, 32], bf16)
            nc.vector.tensor_scalar(ogrh[:], zz[:], oghf[:], None, op0=OP.add)
            ogrl = const.tile([G_PAD, 32], bf16)
            nc.vector.tensor_scalar(ogrl[:], zz[:], oglf[:], None, op0=OP.add)

        # ---- pass 2: out = og.T @ A_T, 512 nodes per matmul ----
        # block b (= r*NQ + q) -> bank-tile q, partition band 32*r
        NQ = (n_ob + 2) // 3
        with tc.tile_pool(name="ps_e", bufs=4, space="PSUM") as ps_e:
            for q in range(NQ):
                e_ps = ps_e.tile([96, OB], f32)
                for r in range(3):
                    b = r * NQ + q
                    if b >= n_ob:
                        continue
                    for w, ogx in ((0, ogrh), (1, ogrl)):
                        nc.tensor.matmul(
                            e_ps[32 * r:32 * r + 32, :], lhsT=ogx[:],
                            rhs=at_sb[:, b * OB:(b + 1) * OB],
                            start=(w == 0), stop=(w == 1),
                        )
                dst = es2[:, q * OB:(q + 1) * OB]
                if q % 2 == 0:
                    nc.vector.tensor_copy(dst, e_ps[:])
                else:
                    nc.scalar.copy(dst, e_ps[:])
            for r in range(3):
                nb = min(NQ, n_ob - r * NQ)
                if nb <= 0:
                    continue
                nc.sync.dma_start(
                    out_d[r * NQ:r * NQ + nb, :].rearrange("a i -> (a i)"),
                    es2[32 * r:32 * r + 1, 0:nb * OB],
                )

    nc.compile()
    return nc


def _shard(batch):
    """Graph-aligned split of nodes across cores, balanced by node count."""
    n = batch.shape[0]
    counts = np.bincount(batch, minlength=G_TOTAL).astype(np.int64)
    bounds = np.concatenate([[0], np.cumsum(counts)])
    gsplit = [0]
    for k in range(1, N_CORES):
        t = k * n // N_CORES
        g = int(np.searchsorted(bounds, t))
        if g > 0 and abs(int(bounds[g - 1]) - t) < abs(int(bounds[g]) - t):
            g -= 1
        g = min(max(g, gsplit[-1]), G_TOTAL)
        gsplit.append(g)
    gsplit.append(G_TOTAL)
    return counts, bounds, gsplit


def kernel(**inputs):
    import ml_dtypes
    from concourse.bass_utils import run_bass_kernel_spmd

    bf16 = ml_dtypes.bfloat16
    x = np.ascontiguousarray(np.asarray(inputs["x"], dtype=np.float32))
    batch = np.asarray(inputs["batch"]).astype(np.int64)
    W1 = np.asarray(inputs["W1"], dtype=np.float32)
    b1 = np.asarray(inputs["b1"], dtype=np.float32)
    W2 = np.asarray(inputs["W2"], dtype=np.float32)
    b2 = np.asarray(inputs["b2"], dtype=np.float32)
    Wc = np.asarray(inputs["Wc"], dtype=np.float32).reshape(H, 1)
    bc = np.asarray(inputs["bc"], dtype=np.float32).reshape(1)

    n = batch.shape[0]
    counts, bounds, gsplit = _shard(batch)
    node_cnt = [int(bounds[gsplit[k + 1]] - bounds[gsplit[k]]) for k in range(N_CORES)]
    pad_unit = np.lcm(CHUNK * XB, OB)  # DMA-group and expansion-block aligned
    nodes_pad = int(-(-max(node_cnt) // pad_unit) * pad_unit)
    assert nodes_pad % OB == 0
    assert max(gsplit[k + 1] - gsplit[k] for k in range(N_CORES)) <= G_PAD

    key = nodes_pad
    if key not in _CACHE:
        _CACHE[key] = _build(nodes_pad)
    nc = _CACHE[key]

    cpk = np.zeros((128, CPK), dtype=np.float32)
    cpk[:, C_ID:C_ID + 128] = np.eye(128, dtype=np.float32)
    cpk[:, C_W1:C_W1 + H] = W1
    cpk[:, C_W2A:C_W2A + H] = W2[0:128]
    cpk[:, C_W2B:C_W2B + H] = W2[128:256]
    cpk[:, C_WC] = Wc[0:128, 0]
    cpk[:, C_WC + 1] = Wc[128:256, 0]
    cpk[:, C_BC] = bc[0]
    cpk[:, C_IO] = np.arange(128, dtype=np.float32)
    cpk[0:G_PAD, C_B1:C_B1 + H] = b1
    cpk[0:G_PAD, C_B2:C_B2 + H] = b2

    n_groups = nodes_pad // (CHUNK * XB)
    in_maps = []
    for k in range(N_CORES):
        gs, ge = gsplit[k], gsplit[k + 1]
        ns, ne = int(bounds[gs]), int(bounds[ge])
        cnt = ne - ns
        bt = np.full(nodes_pad, G_PAD - 1, dtype=np.float32)
        bt[:cnt] = (batch[ns:ne] - gs).astype(np.float32)
        xh = x[ns:ne].astype(bf16)
        xl = (x[ns:ne] - xh.astype(np.float32)).astype(bf16)
        xbp = np.zeros((nodes_pad, ROW), dtype=bf16)
        xbp[:cnt, :D] = xh
        xbp[:cnt, D:2 * D] = xl
        xbp[:, 2 * D + 1] = bt.astype(bf16)  # high half of an f32 via bitcast
        # shuffle to (group, partition, chunk-in-group, row) DMA order
        xbp = np.ascontiguousarray(
            xbp.reshape(n_groups, XB, CHUNK, ROW).transpose(0, 2, 1, 3)
        ).reshape(nodes_pad, ROW)
        cpkk = cpk.copy()
        cpkk[gsplit[k + 1] - gs:G_PAD, C_NC] = 0.0
        cpkk[0:ge - gs, C_NC] = counts[gs:ge].astype(np.float32)
        in_maps.append({
            "xb": xbp,
            "btall": np.ascontiguousarray(bt.astype(bf16).reshape(1, nodes_pad)),
            "cpk": cpkk,
            "iotab": np.ascontiguousarray(
                np.broadcast_to(np.arange(G_PAD, dtype=np.float32),
                                (128, G_PAD)).astype(bf16)),
        })

    res = run_bass_kernel_spmd(nc, in_maps, core_ids=list(range(N_CORES)))
    outs = []
    for k in range(N_CORES):
        o = res.results[k]["out"].reshape(-1)
        outs.append(o[: node_cnt[k]])
    return np.concatenate(outs).reshape(n, 1).astype(np.float32)



# revision 9
# speedup vs baseline: 1.0553x; 1.0553x over previous
"""Trainium2 Bass kernel for nn_Confidence_Score (gnn_message_passing).

Math: with S_g = sum of x over nodes of graph g and n_g = node count,
every node of graph g has identical activations:
    h1_g = relu(S_g @ W1 + b1)
    h2_g = relu((n_g * h1_g) @ W2 + b2)
    c_g  = h2_g @ Wc + bc ;  out_node = sp/(1+sp), sp = softplus(c_g)

Design (v2):
  - x is shipped transposed [128 d, nodes] in fp16 (rel err ~5e-4 vs
    the 2e-2 gate).  Graphs are padded to 8-column blocks; columns are
    interleaved so a 3-level pairwise fp16 add tree on the vector
    engine (scalar_tensor_tensor, 4x DVE mode) yields per-block sums
    B [128 d, nblk] with contiguous access patterns.
  - B tiles are re-oriented with DMA transpose (SBUF->SBUF XBAR, free)
    into B_T [128 blk, 128 d]; one PE matmul per tile against a
    host-sent 0/1 block->graph one-hot M (fp8) accumulates
    S_T [128 d, 72 g] in PSUM.  No per-chunk weight reloads.
  - MLP runs fully transposed: lhsT are the (stationary) weights,
    biases are preloaded into PSUM at program start via outer(b, n)
    rank-1 matmuls, relu/scale are single tensor_scalar ops, the Wc
    contraction is two rank-128 matmuls into a [1, 72] row, softplus
    is the native activation (table preloaded at t=0).
  - Output expansion: out[node] = og[graph(node)] via a per-partition
    window gather: node layout [128 p, NC2] with each row touching at
    most J graphs; masks (bt_rel == j) and window og-selects (ttr
    against prebuilt one-hots) combine in 3 vector ops.  Replaces the
    baseline's 50 expansion matmuls + A_T build + broadcast DMA.

Sharding: graph-aligned contiguous node ranges balanced by node count,
one range per core (8 cores); weights replicated; no collectives.
"""

import os
import sys

for _p in ("/root/.axon_site", "/root/.axon_site/_ro/trn_rl_repo",
           "/root/.axon_site/_ro/pypackages", "/opt/trn_rl_repo"):
    if os.path.isdir(_p) and _p not in sys.path:
        sys.path.append(_p)

import numpy as np

N_CORES = 8
D = 128
H = 256
G_TOTAL = 512
G_PAD = 72        # max local graphs per core (actual ~66)
BLK = 8           # nodes per sum-block (graph pad granularity)
SL = 2048         # node-columns per DMA slice / add-tree unit
TIL = 1024        # node-columns per B_T tile (= 128 blocks)

# wk const packing (fp16, [128, WKC]) column offsets
C_W1 = 0          # W1 [128, 256] (lhsT halves at 0 and 128)
C_W2 = 256        # W2 chunks [h-half, k-half] at 256,384,512,640
C_WC = 768        # Wc as 2 cols (rows 0:128 -> col 768, 128:256 -> 769)
WKC = 770

_CACHE = {}


def _build(nodes_pad, nc2, nj):
    """Single-core Bass program; shapes uniform across cores."""
    from contextlib import ExitStack

    import concourse.bacc as bacc
    import concourse.mybir as mybir
    import concourse.tile as tile

    f32 = mybir.dt.float32
    fp16 = mybir.dt.float16
    fp8 = mybir.dt.float8e4
    AF = mybir.ActivationFunctionType
    OP = mybir.AluOpType

    assert nodes_pad % TIL == 0
    nt = nodes_pad // TIL                      # B_T tiles / lvl2 matmuls
    nblk = nodes_pad // BLK
    slices = []
    off = 0
    while off < nodes_pad:
        ln = min(SL, nodes_pad - off)
        slices.append((off, ln))
        off += ln

    nc = bacc.Bacc("TRN2", target_bir_lowering=False, debug=False)

    xt_d = nc.dram_tensor("xt", [128, nodes_pad], fp16, kind="ExternalInput").ap()
    m8_d = nc.dram_tensor("m8", [128, nt * G_PAD], fp8, kind="ExternalInput").ap()
    wk_d = nc.dram_tensor("wk", [128, WKC], fp16, kind="ExternalInput").ap()
    aux_d = nc.dram_tensor("aux", [128, G_PAD + nc2], fp16, kind="ExternalInput").ap()
    rows_d = nc.dram_tensor("rows", [1, 4 * 128 + 2 * G_PAD], fp16,
                            kind="ExternalInput").ap()
    gpj_d = nc.dram_tensor("gpj", [128, nj], f32, kind="ExternalInput").ap()
    bc_d = nc.dram_tensor("bcv", [1, 1], f32, kind="ExternalInput").ap()
    out_d = nc.dram_tensor("out", [128, nc2], f32, kind="ExternalOutput").ap()
    og_d = nc.dram_tensor("ogx", [1, G_PAD], fp16, kind="Internal").ap()

    with tile.TileContext(nc) as tc, ExitStack() as ctx:
        const = ctx.enter_context(tc.tile_pool(name="const", bufs=1))
        work = ctx.enter_context(tc.tile_pool(name="work", bufs=1))
        psum = ctx.enter_context(tc.tile_pool(name="psum", bufs=1, space="PSUM"))

        # ---- constants (scalar queue) ----
        wk = const.tile([128, WKC], fp16)
        nc.scalar.dma_start(wk[:], wk_d[:])
        m8 = const.tile([128, nt * G_PAD], fp8)
        nc.scalar.dma_start(m8[:], m8_d[:])
        aux = const.tile([128, G_PAD + nc2], fp16)
        nc.scalar.dma_start(aux[:], aux_d[:])
        rows = const.tile([1, 4 * 128 + 2 * G_PAD], fp16)
        nc.scalar.dma_start(rows[:], rows_d[:])
        gpj = const.tile([128, nj], f32)
        nc.scalar.dma_start(gpj[:], gpj_d[:])
        bcv = const.tile([1, 1], f32)
        nc.scalar.dma_start(bcv[:], bc_d[:])
        iota_f = aux[:, 0:G_PAD]
        bt_rel = aux[:, G_PAD:G_PAD + nc2]
        b1a = rows[0:1, 0:128]
        b1b = rows[0:1, 128:256]
        b2a = rows[0:1, 256:384]
        b2b = rows[0:1, 384:512]
        n_row = rows[0:1, 512:512 + G_PAD]
        one_row = rows[0:1, 512 + G_PAD:512 + 2 * G_PAD]

        # n broadcast across partitions (for S scaling)
        nb = const.tile([128, G_PAD], fp16)
        nc.gpsimd.dma_start(nb[:], rows_d[0:1, 512:512 + G_PAD]
                            .to_broadcast((128, G_PAD)))

        # warm the exp/ln activation table during the DMA window
        warm = work.tile([1, 2], f32)
        nc.vector.memset(warm[:], 0.0)
        nc.scalar.activation(warm[:, 0:1], warm[:, 0:1], AF.Exp)
        nc.scalar.activation(warm[:, 1:2], warm[:, 1:2], AF.Ln, bias=1.0)

        # bias preloads into PSUM (rank-1, run early; mm1/mm2 accumulate)
        h1a_ps = psum.tile([128, G_PAD], f32)
        h1b_ps = psum.tile([128, G_PAD], f32)
        h2a_ps = psum.tile([128, G_PAD], f32)
        h2b_ps = psum.tile([128, G_PAD], f32)
        nc.tensor.matmul(h1a_ps[:], lhsT=b1a, rhs=n_row, start=True, stop=False)
        nc.tensor.matmul(h1b_ps[:], lhsT=b1b, rhs=n_row, start=True, stop=False)
        nc.tensor.matmul(h2a_ps[:], lhsT=b2a, rhs=one_row, start=True, stop=False)
        nc.tensor.matmul(h2b_ps[:], lhsT=b2b, rhs=one_row, start=True, stop=False)

        # prebuilt pass-2 masks and window one-hots (hidden under DMA)
        masks = work.tile([128, nj * nc2], fp16)
        iseq = work.tile([128, nj * G_PAD], fp16)
        for j in range(nj):
            nc.vector.tensor_scalar(
                masks[:, j * nc2:(j + 1) * nc2], bt_rel, float(j), None,
                op0=OP.is_equal)
            nc.vector.tensor_scalar(
                iseq[:, j * G_PAD:(j + 1) * G_PAD], iota_f,
                gpj[:, j:j + 1], None, op0=OP.is_equal)

        # ---- pass 1: x DMA + fp16 add tree + transpose + block matmuls ----
        bsum = work.tile([128, nblk], fp16)
        st_ps = psum.tile([128, G_PAD], f32)
        bt_tiles = work.tile([128, nt * 128], fp16)
        t_done = 0
        with (
            tc.tile_pool(name="xp", bufs=3) as xpool,
            tc.tile_pool(name="scr", bufs=2) as spool,
        ):
            for si, (off, ln) in enumerate(slices):
                xs = xpool.tile([128, SL], fp16, tag="xs")
                nc.sync.dma_start(xs[:, 0:ln], xt_d[:, off:off + ln])
                h = ln // 2
                s1 = spool.tile([128, SL // 2], fp16, tag="s1")
                nc.vector.scalar_tensor_tensor(
                    s1[:, 0:h], xs[:, 0:h], 0.0, xs[:, h:ln],
                    op0=OP.bypass, op1=OP.add)
                q = ln // 4
                s2 = spool.tile([128, SL // 4], fp16, tag="s2")
                nc.vector.scalar_tensor_tensor(
                    s2[:, 0:q], s1[:, 0:q], 0.0, s1[:, q:2 * q],
                    op0=OP.bypass, op1=OP.add)
                e = ln // 8
                bo = off // BLK
                nc.vector.scalar_tensor_tensor(
                    bsum[:, bo:bo + e], s2[:, 0:e // 1], 0.0, s2[:, e:2 * e],
                    op0=OP.bypass, op1=OP.add)
                # B_T tiles fully covered by bsum so far
                while (t_done + 1) * 128 <= bo + e:
                    t = t_done
                    nc.scalar.dma_start_transpose(
                        bt_tiles[:, t * 128:(t + 1) * 128],
                        bsum[:, t * 128:(t + 1) * 128])
                    nc.tensor.matmul(
                        st_ps[:], lhsT=bt_tiles[:, t * 128:(t + 1) * 128],
                        rhs=m8[:, t * G_PAD:(t + 1) * G_PAD],
                        start=(t == 0), stop=(t == nt - 1))
                    t_done += 1
        assert t_done == nt

        # ---- per-graph MLP (transposed; graphs on free axis) ----
        st16 = work.tile([128, G_PAD], fp16)
        nc.vector.tensor_tensor(st16[:], st_ps[:], nb[:], op=OP.mult)

        nc.tensor.matmul(h1a_ps[:], lhsT=wk[:, C_W1:C_W1 + 128], rhs=st16[:],
                         start=False, stop=True)
        nc.tensor.matmul(h1b_ps[:], lhsT=wk[:, C_W1 + 128:C_W1 + 256],
                         rhs=st16[:], start=False, stop=True)
        h1n = work.tile([128, 2 * G_PAD], fp16)
        nc.vector.tensor_scalar_max(h1n[:, 0:G_PAD], h1a_ps[:], 0.0)
        nc.vector.tensor_scalar_max(h1n[:, G_PAD:2 * G_PAD], h1b_ps[:], 0.0)

        nc.tensor.matmul(h2a_ps[:], lhsT=wk[:, C_W2:C_W2 + 128],
                         rhs=h1n[:, 0:G_PAD], start=False, stop=False)
        nc.tensor.matmul(h2a_ps[:], lhsT=wk[:, C_W2 + 256:C_W2 + 384],
                         rhs=h1n[:, G_PAD:2 * G_PAD], start=False, stop=True)
        nc.tensor.matmul(h2b_ps[:], lhsT=wk[:, C_W2 + 128:C_W2 + 256],
                         rhs=h1n[:, 0:G_PAD], start=False, stop=False)
        nc.tensor.matmul(h2b_ps[:], lhsT=wk[:, C_W2 + 384:C_W2 + 512],
                         rhs=h1n[:, G_PAD:2 * G_PAD], start=False, stop=True)
        h2n = work.tile([128, 2 * G_PAD], fp16)
        nc.vector.tensor_scalar_max(h2n[:, 0:G_PAD], h2a_ps[:], 0.0)
        nc.vector.tensor_scalar_max(h2n[:, G_PAD:2 * G_PAD], h2b_ps[:], 0.0)

        c_ps = psum.tile([1, G_PAD], f32)
        nc.tensor.matmul(c_ps[:], lhsT=wk[:, C_WC:C_WC + 1],
                         rhs=h2n[:, 0:G_PAD], start=True, stop=False)
        nc.tensor.matmul(c_ps[:], lhsT=wk[:, C_WC + 1:C_WC + 2],
                         rhs=h2n[:, G_PAD:2 * G_PAD], start=False, stop=True)

        # og = sp/(1+sp) = 1 - 1/(1+sp), sp = softplus(c + bc)
        # softplus(cc) = relu(cc) + ln(1 + exp(-|cc|))  (exp+ln: one table)
        cc = work.tile([1, G_PAD], f32)
        nc.vector.tensor_scalar(cc[:], c_ps[:], bcv[0:1, 0:1], None, op0=OP.add)
        nab = work.tile([1, G_PAD], f32)
        nc.vector.scalar_tensor_tensor(nab[:], cc[:], -1.0, cc[:],
                                       op0=OP.mult, op1=OP.min)
        ex = work.tile([1, G_PAD], f32)
        nc.scalar.activation(ex[:], nab[:], AF.Exp)
        l1 = work.tile([1, G_PAD], f32)
        nc.scalar.activation(l1[:], ex[:], AF.Ln, bias=1.0)
        sp = work.tile([1, G_PAD], f32)
        nc.vector.scalar_tensor_tensor(sp[:], cc[:], 0.0, l1[:],
                                       op0=OP.max, op1=OP.add)
        t1 = work.tile([1, G_PAD], f32)
        nc.vector.tensor_scalar_add(t1[:], sp[:], 1.0)
        rcp = work.tile([1, G_PAD], f32)
        nc.vector.reciprocal(rcp[:], t1[:])
        og16 = work.tile([1, G_PAD], fp16)
        nc.vector.tensor_scalar(og16[:], rcp[:], -1.0, 1.0,
                                op0=OP.mult, op1=OP.add)

        # ---- pass 2: out[p, c] = og[gp[p] + bt_rel[p, c]] ----
        ogb = work.tile([128, G_PAD], fp16)
        nc.scalar.dma_start(og_d[:], og16[:])
        nc.scalar.dma_start(ogb[:], og_d[0:1, :].to_broadcast((128, G_PAD)))
        scr = work.tile([128, G_PAD], fp16)
        ogsel = work.tile([128, nj], f32)
        for j in range(nj):
            nc.vector.scalar_tensor_tensor(
                scr[:], ogb[:], 0.0, iseq[:, j * G_PAD:(j + 1) * G_PAD],
                op0=OP.bypass, op1=OP.mult,
                accum_out=ogsel[:, j:j + 1])
        outT = work.tile([128, nc2], f32)
        if nj == 1:
            nc.vector.tensor_scalar(outT[:], masks[:, 0:nc2],
                                    ogsel[:, 0:1], None, op0=OP.mult)
        else:
            term = work.tile([128, nc2], fp16)
            nc.vector.tensor_scalar(term[:], masks[:, nc2:2 * nc2],
                                    ogsel[:, 1:2], None, op0=OP.mult)
            for j in range(2, nj):
                t2 = work.tile([128, nc2], fp16, tag=f"t{j}")
                nc.vector.tensor_scalar(t2[:], masks[:, j * nc2:(j + 1) * nc2],
                                        ogsel[:, j:j + 1], None, op0=OP.mult)
                nc.vector.tensor_tensor(term[:], term[:], t2[:], op=OP.add)
            nc.vector.scalar_tensor_tensor(
                outT[:], masks[:, 0:nc2], ogsel[:, 0:1], term[:],
                op0=OP.mult, op1=OP.add)
        nc.sync.dma_start(out_d[:], outT[:])

    nc.compile()
    return nc


def _shard(batch):
    """Graph-aligned split of nodes across cores, balanced by node count."""
    n = batch.shape[0]
    counts = np.bincount(batch, minlength=G_TOTAL).astype(np.int64)
    bounds = np.concatenate([[0], np.cumsum(counts)])
    gsplit = [0]
    for k in range(1, N_CORES):
        t = k * n // N_CORES
        g = int(np.searchsorted(bounds, t))
        if g > 0 and abs(int(bounds[g - 1]) - t) < abs(int(bounds[g]) - t):
            g -= 1
        g = min(max(g, gsplit[-1]), G_TOTAL)
        gsplit.append(g)
    gsplit.append(G_TOTAL)
    return counts, bounds, gsplit


def kernel(**inputs):
    from concourse.bass_utils import run_bass_kernel_spmd

    x = np.ascontiguousarray(np.asarray(inputs["x"], dtype=np.float32))
    batch = np.asarray(inputs["batch"]).astype(np.int64)
    W1 = np.asarray(inputs["W1"], dtype=np.float32)
    b1 = np.asarray(inputs["b1"], dtype=np.float32)
    W2 = np.asarray(inputs["W2"], dtype=np.float32)
    b2 = np.asarray(inputs["b2"], dtype=np.float32)
    Wc = np.asarray(inputs["Wc"], dtype=np.float32).reshape(H, 1)
    bc = np.asarray(inputs["bc"], dtype=np.float32).reshape(1)

    n = batch.shape[0]
    counts, bounds, gsplit = _shard(batch)
    node_cnt = [int(bounds[gsplit[k + 1]] - bounds[gsplit[k]]) for k in range(N_CORES)]
    ng_local = [gsplit[k + 1] - gsplit[k] for k in range(N_CORES)]
    assert max(ng_local) <= G_PAD

    # pass-1 padded length: graphs padded to BLK, total to TIL
    padded_len = []
    for k in range(N_CORES):
        gs, ge = gsplit[k], gsplit[k + 1]
        padded_len.append(int(np.sum(-(-counts[gs:ge] // BLK) * BLK)))
    nodes_pad = int(-(-max(padded_len) // TIL) * TIL)
    # pass-2 padded length (dense node layout, partition-major)
    nc2 = int(-(-max(node_cnt) // 128))

    # J = max graphs touched by one output row of nc2 nodes
    nj = 1
    core_meta = []
    for k in range(N_CORES):
        gs, ge = gsplit[k], gsplit[k + 1]
        gl = (batch[bounds[gs]:bounds[ge]] - gs).astype(np.int64)
        gl_pad = np.zeros(128 * nc2, dtype=np.int64)
        gl_pad[:node_cnt[k]] = gl
        gl_pad[node_cnt[k]:] = gl[-1] if node_cnt[k] else 0
        gl2 = gl_pad.reshape(128, nc2)
        gp = gl2[:, 0]
        rel = gl2 - gp[:, None]
        nj = max(nj, int(rel.max()) + 1)
        core_meta.append((gs, ge, gl2, gp, rel))

    key = (nodes_pad, nc2, nj)
    if key not in _CACHE:
        _CACHE[key] = _build(nodes_pad, nc2, nj)
    nc = _CACHE[key]

    nt = nodes_pad // TIL
    wk = np.zeros((128, WKC), dtype=np.float16)
    wk[:, C_W1:C_W1 + H] = W1
    for hh in range(2):
        for kk in range(2):
            wk[:, C_W2 + (2 * hh + kk) * 128:C_W2 + (2 * hh + kk + 1) * 128] = \
                W2[hh * 128:(hh + 1) * 128, kk * 128:(kk + 1) * 128]
    wk[:, C_WC] = Wc[0:128, 0]
    wk[:, C_WC + 1] = Wc[128:256, 0]

    # slice interleave maps: for a slice of ln columns, block i elem k
    # sits at column i + k * (ln // 8)
    in_maps = []
    for k in range(N_CORES):
        gs, ge, gl2, gp, rel = core_meta[k]
        ns, ne = int(bounds[gs]), int(bounds[ge])
        cnt = ne - ns
        ngl = ge - gs

        # padded node index list: graph-aligned groups of BLK (-1 = zero)
        idx = np.full(nodes_pad, -1, dtype=np.int64)
        pos = 0
        for g in range(gs, ge):
            c0, c1 = int(bounds[g]) - ns, int(bounds[g + 1]) - ns
            m = c1 - c0
            idx[pos:pos + m] = np.arange(c0, c1)
            pos += -(-m // BLK) * BLK
        # graph id per block
        blk_gid = np.full(nodes_pad // BLK, -1, dtype=np.int64)
        bpos = 0
        for g in range(gs, ge):
            nb_g = -(-int(counts[g]) // BLK)
            blk_gid[bpos:bpos + nb_g] = g - gs
            bpos += nb_g

        # interleave within slices
        perm = np.empty(nodes_pad, dtype=np.int64)
        off = 0
        while off < nodes_pad:
            ln = min(SL, nodes_pad - off)
            e = ln // BLK
            src = idx[off:off + ln].reshape(e, BLK)      # [block, elem]
            perm[off:off + ln] = src.T.reshape(-1)       # col i + k*e
            off += ln

        xsrc = np.concatenate([x[ns:ne], np.zeros((1, D), np.float32)])
        xt = np.ascontiguousarray(
            xsrc[perm].T.astype(np.float16))             # [-1] -> zero row

        m8 = np.zeros((128, nt * G_PAD), dtype=np.float16)
        for t in range(nt):
            gid = blk_gid[t * 128:(t + 1) * 128]
            ok = gid >= 0
            m8[np.nonzero(ok)[0], t * G_PAD + gid[ok]] = 1.0
        import ml_dtypes
        m8 = m8.astype(ml_dtypes.float8_e4m3fn)

        aux = np.zeros((128, G_PAD + nc2), dtype=np.float16)
        aux[:, 0:G_PAD] = np.arange(G_PAD, dtype=np.float16)[None, :]
        aux[:, G_PAD:] = rel.astype(np.float16)

        rows = np.zeros((1, 4 * 128 + 2 * G_PAD), dtype=np.float16)
        rows[0, 0:128] = b1[0:128]
        rows[0, 128:256] = b1[128:256]
        rows[0, 256:384] = b2[0:128]
        rows[0, 384:512] = b2[128:256]
        rows[0, 512 + ngl:512 + G_PAD] = 0.0
        rows[0, 512:512 + ngl] = counts[gs:ge].astype(np.float16)
        rows[0, 512 + G_PAD:512 + 2 * G_PAD] = 1.0

        gpj = np.zeros((128, nj), dtype=np.float32)
        for j in range(nj):
            gpj[:, j] = gp + j

        in_maps.append({
            "xt": xt,
            "m8": m8,
            "wk": wk,
            "aux": aux,
            "rows": rows,
            "gpj": gpj,
            "bcv": np.full((1, 1), bc[0], dtype=np.float32),
        })

    res = run_bass_kernel_spmd(nc, in_maps, core_ids=list(range(N_CORES)))
    outs = []
    for k in range(N_CORES):
        o = res.results[k]["out"].reshape(-1)
        outs.append(o[: node_cnt[k]])
    return np.concatenate(outs).reshape(n, 1).astype(np.float32)


# revision 14
# speedup vs baseline: 1.5001x; 1.4214x over previous
"""Trainium2 Bass kernel for nn_Confidence_Score (gnn_message_passing).

Math: with S_g = sum of x over nodes of graph g and n_g = node count,
every node of graph g has identical activations:
    h1_g = relu(S_g @ W1 + b1)
    h2_g = relu((n_g * h1_g) @ W2 + b2)
    c_g  = h2_g @ Wc + bc ;  out_node = sp/(1+sp), sp = softplus(c_g)

Design (v2):
  - x is shipped transposed [128 d, nodes] in fp16 (rel err ~5e-4 vs
    the 2e-2 gate).  Graphs are padded to 8-column blocks; columns are
    interleaved so a 3-level pairwise fp16 add tree on the vector
    engine (scalar_tensor_tensor, 4x DVE mode) yields per-block sums
    B [128 d, nblk] with contiguous access patterns.
  - B tiles are re-oriented with DMA transpose (SBUF->SBUF XBAR, free)
    into B_T [128 blk, 128 d]; one PE matmul per tile against a
    host-sent 0/1 block->graph one-hot M (fp8) accumulates
    S_T [128 d, 72 g] in PSUM.  No per-chunk weight reloads.
  - MLP runs fully transposed: lhsT are the (stationary) weights,
    biases are preloaded into PSUM at program start via outer(b, n)
    rank-1 matmuls, relu/scale are single tensor_scalar ops, the Wc
    contraction is two rank-128 matmuls into a [1, 72] row, softplus
    is the native activation (table preloaded at t=0).
  - Output expansion: out[node] = og[graph(node)] via a per-partition
    window gather: node layout [128 p, NC2] with each row touching at
    most J graphs; masks (bt_rel == j) and window og-selects (ttr
    against prebuilt one-hots) combine in 3 vector ops.  Replaces the
    baseline's 50 expansion matmuls + A_T build + broadcast DMA.

Sharding: graph-aligned contiguous node ranges balanced by node count,
one range per core (8 cores); weights replicated; no collectives.
"""

import os
import sys

for _p in ("/root/.axon_site", "/root/.axon_site/_ro/trn_rl_repo",
           "/root/.axon_site/_ro/pypackages", "/opt/trn_rl_repo"):
    if os.path.isdir(_p) and _p not in sys.path:
        sys.path.append(_p)

import numpy as np

N_CORES = 8
D = 128
H = 256
G_TOTAL = 512
G_PAD = 72        # max local graphs per core (actual ~66)
BLK = 8           # nodes per sum-block (graph pad granularity)
SL = 2048         # node-columns per DMA slice / add-tree unit
TIL = 1024        # node-columns per B_T tile (= 128 blocks)

# wk const packing (fp16, [128, WKC]) column offsets
C_W1 = 0          # W1 [128, 256] (lhsT halves at 0 and 128)
C_W2 = 256        # W2 chunks [h-half, k-half] at 256,384,512,640
C_WC = 768        # Wc as 2 cols (rows 0:128 -> col 768, 128:256 -> 769)
WKC = 770

_CACHE = {}


def _act_set_id(nc, AF):
    """Index of the activation-table set holding both Exp and Ln."""
    from concourse.hw_specs import get_activation_tables
    tabs = get_activation_tables(nc.m.arch)
    for i, (name, funcs) in enumerate(tabs.items()):
        if AF.Exp in funcs and AF.Ln in funcs:
            return i
    raise RuntimeError("no exp+ln activation table")


def _build(nodes_pad, nc2, nj):
    """Single-core Bass program; shapes uniform across cores."""
    from contextlib import ExitStack

    import concourse.bacc as bacc
    import concourse.mybir as mybir
    import concourse.tile as tile

    f32 = mybir.dt.float32
    fp16 = mybir.dt.float16
    fp8 = mybir.dt.float8e4
    AF = mybir.ActivationFunctionType
    OP = mybir.AluOpType

    assert nodes_pad % TIL == 0
    nt = nodes_pad // TIL                      # B_T tiles / lvl2 matmuls
    nblk = nodes_pad // BLK
    slices = []
    off = 0
    while off < nodes_pad:
        ln = min(SL, nodes_pad - off)
        slices.append((off, ln))
        off += ln

    nc = bacc.Bacc("TRN2", target_bir_lowering=False, debug=False)

    xt_d = nc.dram_tensor("xt", [128, nodes_pad], fp16, kind="ExternalInput").ap()
    m8_d = nc.dram_tensor("m8", [128, nt * G_PAD], fp8, kind="ExternalInput").ap()
    wk_d = nc.dram_tensor("wk", [128, WKC], fp16, kind="ExternalInput").ap()
    aux_d = nc.dram_tensor("aux", [128, G_PAD + nc2], fp16, kind="ExternalInput").ap()
    rows_d = nc.dram_tensor("rows", [1, 4 * 128 + 2 * G_PAD], fp16,
                            kind="ExternalInput").ap()
    gpj_d = nc.dram_tensor("gpj", [128, nj], f32, kind="ExternalInput").ap()
    bc_d = nc.dram_tensor("bcv", [1, 1], f32, kind="ExternalInput").ap()
    out_d = nc.dram_tensor("out", [128, nc2], f32, kind="ExternalOutput").ap()
    og_d = nc.dram_tensor("ogx", [1, G_PAD], fp16, kind="Internal").ap()

    with tile.TileContext(nc) as tc, ExitStack() as ctx:
        const = ctx.enter_context(tc.tile_pool(name="const", bufs=1))
        work = ctx.enter_context(tc.tile_pool(name="work", bufs=1))
        psum = ctx.enter_context(tc.tile_pool(name="psum", bufs=1, space="PSUM"))

        # ---- constants (scalar queue) ----
        wk = const.tile([128, WKC], fp16)
        nc.scalar.dma_start(wk[:], wk_d[:])
        m8 = const.tile([128, nt * G_PAD], fp8)
        nc.scalar.dma_start(m8[:], m8_d[:])
        aux = const.tile([128, G_PAD + nc2], fp16)
        nc.scalar.dma_start(aux[:], aux_d[:])
        rows = const.tile([1, 4 * 128 + 2 * G_PAD], fp16)
        nc.scalar.dma_start(rows[:], rows_d[:])
        gpj = const.tile([128, nj], f32)
        nc.scalar.dma_start(gpj[:], gpj_d[:])
        bcv = const.tile([1, 1], f32)
        nc.scalar.dma_start(bcv[:], bc_d[:])
        iota_f = aux[:, 0:G_PAD]
        bt_rel = aux[:, G_PAD:G_PAD + nc2]
        b1a = rows[0:1, 0:128]
        b1b = rows[0:1, 128:256]
        b2a = rows[0:1, 256:384]
        b2b = rows[0:1, 384:512]
        n_row = rows[0:1, 512:512 + G_PAD]
        one_row = rows[0:1, 512 + G_PAD:512 + 2 * G_PAD]

        # n broadcast across partitions (for S scaling)
        nb = const.tile([128, G_PAD], fp16)
        nc.gpsimd.dma_start(nb[:], rows_d[0:1, 512:512 + G_PAD]
                            .to_broadcast((128, G_PAD)))

        # preload the combined exp+ln activation table during the DMA window
        nc.scalar.add_instruction(mybir.InstLoadActFuncSet(
            name=nc.get_next_instruction_name(), ins=[], outs=[],
            act_func_set_id=_act_set_id(nc, AF)))

        # bias preloads into PSUM (rank-1, run early; mm1/mm2 accumulate)
        h1a_ps = psum.tile([128, G_PAD], f32)
        h1b_ps = psum.tile([128, G_PAD], f32)
        h2a_ps = psum.tile([128, G_PAD], f32)
        h2b_ps = psum.tile([128, G_PAD], f32)
        nc.tensor.matmul(h1a_ps[:], lhsT=b1a, rhs=n_row, start=True, stop=False)
        nc.tensor.matmul(h1b_ps[:], lhsT=b1b, rhs=n_row, start=True, stop=False)
        nc.tensor.matmul(h2a_ps[:], lhsT=b2a, rhs=one_row, start=True, stop=False)
        nc.tensor.matmul(h2b_ps[:], lhsT=b2b, rhs=one_row, start=True, stop=False)

        # prebuilt pass-2 masks and window one-hots (hidden under DMA)
        masks = work.tile([128, nj * nc2], fp16)
        iseq = work.tile([128, nj * G_PAD], fp16)
        for j in range(nj):
            nc.vector.tensor_scalar(
                masks[:, j * nc2:(j + 1) * nc2], bt_rel, float(j), None,
                op0=OP.is_equal)
            nc.vector.tensor_scalar(
                iseq[:, j * G_PAD:(j + 1) * G_PAD], iota_f,
                gpj[:, j:j + 1], None, op0=OP.is_equal)

        # ---- pass 1: x DMA + fp16 add tree + transpose + block matmuls ----
        bsum = work.tile([128, nblk], fp16)
        st_ps = psum.tile([128, G_PAD], f32)
        bt_tiles = work.tile([128, nt * 128], fp16)
        # transpose groups: one blocked XBAR call per group of B_T tiles
        n_grp = 3 if nt >= 6 else 1
        gsz = -(-nt // n_grp)
        grp_bounds = [min(g * gsz, nt) for g in range(n_grp + 1)]
        t_done = 0
        with (
            tc.tile_pool(name="xp", bufs=3) as xpool,
            tc.tile_pool(name="scr", bufs=2) as spool,
        ):
            for si, (off, ln) in enumerate(slices):
                xs = xpool.tile([128, SL], fp16, tag="xs")
                nc.sync.dma_start(xs[:, 0:ln], xt_d[:, off:off + ln])
                h = ln // 2
                s1 = spool.tile([128, SL // 2], fp16, tag="s1")
                nc.vector.tensor_tensor(
                    s1[:, 0:h], xs[:, 0:h], xs[:, h:ln], op=OP.add)
                q = ln // 4
                s2 = spool.tile([128, SL // 4], fp16, tag="s2")
                nc.vector.tensor_tensor(
                    s2[:, 0:q], s1[:, 0:q], s1[:, q:2 * q], op=OP.add)
                e = ln // 8
                bo = off // BLK
                nc.vector.tensor_tensor(
                    bsum[:, bo:bo + e], s2[:, 0:e], s2[:, e:2 * e], op=OP.add)
                # issue transpose groups fully covered by bsum so far
                while t_done < nt and (grp := grp_bounds.index(t_done)) < n_grp \
                        and grp_bounds[grp + 1] * 128 <= bo + e:
                    t0, t1g = grp_bounds[grp], grp_bounds[grp + 1]
                    k = t1g - t0
                    nc.scalar.dma_start_transpose(
                        bt_tiles[:, t0 * 128:t1g * 128]
                        .rearrange("p (k b) -> p k b", k=k),
                        bsum[:, t0 * 128:t1g * 128])
                    for t in range(t0, t1g):
                        nc.tensor.matmul(
                            st_ps[:], lhsT=bt_tiles[:, t * 128:(t + 1) * 128],
                            rhs=m8[:, t * G_PAD:(t + 1) * G_PAD],
                            start=(t == 0), stop=(t == nt - 1))
                    t_done = t1g
        assert t_done == nt, (t_done, nt)

        # ---- per-graph MLP (transposed; graphs on free axis) ----
        st16 = work.tile([128, G_PAD], fp16)
        nc.vector.tensor_tensor(st16[:], st_ps[:], nb[:], op=OP.mult)

        nc.tensor.matmul(h1a_ps[:], lhsT=wk[:, C_W1:C_W1 + 128], rhs=st16[:],
                         start=False, stop=True)
        nc.tensor.matmul(h1b_ps[:], lhsT=wk[:, C_W1 + 128:C_W1 + 256],
                         rhs=st16[:], start=False, stop=True)
        h1n = work.tile([128, 2 * G_PAD], fp16)
        nc.vector.tensor_scalar_max(h1n[:, 0:G_PAD], h1a_ps[:], 0.0)
        nc.vector.tensor_scalar_max(h1n[:, G_PAD:2 * G_PAD], h1b_ps[:], 0.0)

        nc.tensor.matmul(h2a_ps[:], lhsT=wk[:, C_W2:C_W2 + 128],
                         rhs=h1n[:, 0:G_PAD], start=False, stop=False)
        nc.tensor.matmul(h2a_ps[:], lhsT=wk[:, C_W2 + 256:C_W2 + 384],
                         rhs=h1n[:, G_PAD:2 * G_PAD], start=False, stop=True)
        nc.tensor.matmul(h2b_ps[:], lhsT=wk[:, C_W2 + 128:C_W2 + 256],
                         rhs=h1n[:, 0:G_PAD], start=False, stop=False)
        nc.tensor.matmul(h2b_ps[:], lhsT=wk[:, C_W2 + 384:C_W2 + 512],
                         rhs=h1n[:, G_PAD:2 * G_PAD], start=False, stop=True)
        h2n = work.tile([128, 2 * G_PAD], fp16)
        nc.vector.tensor_scalar_max(h2n[:, 0:G_PAD], h2a_ps[:], 0.0)
        nc.vector.tensor_scalar_max(h2n[:, G_PAD:2 * G_PAD], h2b_ps[:], 0.0)

        c_ps = psum.tile([1, G_PAD], f32)
        nc.tensor.matmul(c_ps[:], lhsT=wk[:, C_WC:C_WC + 1],
                         rhs=h2n[:, 0:G_PAD], start=True, stop=False)
        nc.tensor.matmul(c_ps[:], lhsT=wk[:, C_WC + 1:C_WC + 2],
                         rhs=h2n[:, G_PAD:2 * G_PAD], start=False, stop=True)

        # og = sp/(1+sp) = 1 - 1/(1+sp), sp = softplus(c + bc)
        # softplus(cc) = relu(cc) + ln(1 + exp(-|cc|))  (exp+ln: one table)
        cc = work.tile([1, G_PAD], f32)
        nc.vector.tensor_scalar(cc[:], c_ps[:], bcv[0:1, 0:1], None, op0=OP.add)
        nab = work.tile([1, G_PAD], f32)
        nc.vector.scalar_tensor_tensor(nab[:], cc[:], -1.0, cc[:],
                                       op0=OP.mult, op1=OP.min)
        ex = work.tile([1, G_PAD], f32)
        nc.scalar.activation(ex[:], nab[:], AF.Exp)
        l1 = work.tile([1, G_PAD], f32)
        nc.scalar.activation(l1[:], ex[:], AF.Ln, bias=1.0)
        sp = work.tile([1, G_PAD], f32)
        nc.vector.scalar_tensor_tensor(sp[:], cc[:], 0.0, l1[:],
                                       op0=OP.max, op1=OP.add)
        t1 = work.tile([1, G_PAD], f32)
        nc.vector.tensor_scalar_add(t1[:], sp[:], 1.0)
        rcp = work.tile([1, G_PAD], f32)
        nc.vector.reciprocal(rcp[:], t1[:])
        og16 = work.tile([1, G_PAD], fp16)
        nc.vector.tensor_scalar(og16[:], rcp[:], -1.0, 1.0,
                                op0=OP.mult, op1=OP.add)

        # ---- pass 2: out[p, c] = og[gp[p] + bt_rel[p, c]] ----
        ogb = work.tile([128, G_PAD], fp16)
        nc.scalar.dma_start(og_d[:], og16[:])
        nc.scalar.dma_start(ogb[:], og_d[0:1, :].to_broadcast((128, G_PAD)))
        scr = work.tile([128, G_PAD], fp16)
        ogsel = work.tile([128, nj], f32)
        for j in range(nj):
            nc.vector.scalar_tensor_tensor(
                scr[:], ogb[:], 0.0, iseq[:, j * G_PAD:(j + 1) * G_PAD],
                op0=OP.bypass, op1=OP.mult,
                accum_out=ogsel[:, j:j + 1])
        outT = work.tile([128, nc2], f32)
        if nj == 1:
            nc.vector.tensor_scalar(outT[:], masks[:, 0:nc2],
                                    ogsel[:, 0:1], None, op0=OP.mult)
        else:
            term = work.tile([128, nc2], fp16)
            nc.vector.tensor_scalar(term[:], masks[:, nc2:2 * nc2],
                                    ogsel[:, 1:2], None, op0=OP.mult)
            for j in range(2, nj):
                t2 = work.tile([128, nc2], fp16, tag=f"t{j}")
                nc.vector.tensor_scalar(t2[:], masks[:, j * nc2:(j + 1) * nc2],
                                        ogsel[:, j:j + 1], None, op0=OP.mult)
                nc.vector.tensor_tensor(term[:], term[:], t2[:], op=OP.add)
            nc.vector.scalar_tensor_tensor(
                outT[:], masks[:, 0:nc2], ogsel[:, 0:1], term[:],
                op0=OP.mult, op1=OP.add)
        nc.sync.dma_start(out_d[:], outT[:])

    nc.compile()
    return nc


def _shard(batch):
    """Graph-aligned split of nodes across cores, balanced by node count."""
    n = batch.shape[0]
    counts = np.bincount(batch, minlength=G_TOTAL).astype(np.int64)
    bounds = np.concatenate([[0], np.cumsum(counts)])
    gsplit = [0]
    for k in range(1, N_CORES):
        t = k * n // N_CORES
        g = int(np.searchsorted(bounds, t))
        if g > 0 and abs(int(bounds[g - 1]) - t) < abs(int(bounds[g]) - t):
            g -= 1
        g = min(max(g, gsplit[-1]), G_TOTAL)
        gsplit.append(g)
    gsplit.append(G_TOTAL)
    return counts, bounds, gsplit


def kernel(**inputs):
    from concourse.bass_utils import run_bass_kernel_spmd

    x = np.ascontiguousarray(np.asarray(inputs["x"], dtype=np.float32))
    batch = np.asarray(inputs["batch"]).astype(np.int64)
    W1 = np.asarray(inputs["W1"], dtype=np.float32)
    b1 = np.asarray(inputs["b1"], dtype=np.float32)
    W2 = np.asarray(inputs["W2"], dtype=np.float32)
    b2 = np.asarray(inputs["b2"], dtype=np.float32)
    Wc = np.asarray(inputs["Wc"], dtype=np.float32).reshape(H, 1)
    bc = np.asarray(inputs["bc"], dtype=np.float32).reshape(1)

    n = batch.shape[0]
    counts, bounds, gsplit = _shard(batch)
    node_cnt = [int(bounds[gsplit[k + 1]] - bounds[gsplit[k]]) for k in range(N_CORES)]
    ng_local = [gsplit[k + 1] - gsplit[k] for k in range(N_CORES)]
    assert max(ng_local) <= G_PAD

    # pass-1 padded length: graphs padded to BLK, total to TIL
    padded_len = []
    for k in range(N_CORES):
        gs, ge = gsplit[k], gsplit[k + 1]
        padded_len.append(int(np.sum(-(-counts[gs:ge] // BLK) * BLK)))
    nodes_pad = int(-(-max(padded_len) // TIL) * TIL)
    # pass-2 padded length (dense node layout, partition-major)
    nc2 = int(-(-max(node_cnt) // 128))

    # J = max graphs touched by one output row of nc2 nodes
    nj = 1
    core_meta = []
    for k in range(N_CORES):
        gs, ge = gsplit[k], gsplit[k + 1]
        gl = (batch[bounds[gs]:bounds[ge]] - gs).astype(np.int64)
        gl_pad = np.zeros(128 * nc2, dtype=np.int64)
        gl_pad[:node_cnt[k]] = gl
        gl_pad[node_cnt[k]:] = gl[-1] if node_cnt[k] else 0
        gl2 = gl_pad.reshape(128, nc2)
        gp = gl2[:, 0]
        rel = gl2 - gp[:, None]
        nj = max(nj, int(rel.max()) + 1)
        core_meta.append((gs, ge, gl2, gp, rel))

    key = (nodes_pad, nc2, nj)
    if key not in _CACHE:
        _CACHE[key] = _build(nodes_pad, nc2, nj)
    nc = _CACHE[key]

    nt = nodes_pad // TIL
    wk = np.zeros((128, WKC), dtype=np.float16)
    wk[:, C_W1:C_W1 + H] = W1
    for hh in range(2):
        for kk in range(2):
            wk[:, C_W2 + (2 * hh + kk) * 128:C_W2 + (2 * hh + kk + 1) * 128] = \
                W2[hh * 128:(hh + 1) * 128, kk * 128:(kk + 1) * 128]
    wk[:, C_WC] = Wc[0:128, 0]
    wk[:, C_WC + 1] = Wc[128:256, 0]

    # slice interleave maps: for a slice of ln columns, block i elem k
    # sits at column i + k * (ln // 8)
    in_maps = []
    for k in range(N_CORES):
        gs, ge, gl2, gp, rel = core_meta[k]
        ns, ne = int(bounds[gs]), int(bounds[ge])
        cnt = ne - ns
        ngl = ge - gs

        # padded node index list: graph-aligned groups of BLK (-1 = zero)
        idx = np.full(nodes_pad, -1, dtype=np.int64)
        pos = 0
        for g in range(gs, ge):
            c0, c1 = int(bounds[g]) - ns, int(bounds[g + 1]) - ns
            m = c1 - c0
            idx[pos:pos + m] = np.arange(c0, c1)
            pos += -(-m // BLK) * BLK
        # graph id per block
        blk_gid = np.full(nodes_pad // BLK, -1, dtype=np.int64)
        bpos = 0
        for g in range(gs, ge):
            nb_g = -(-int(counts[g]) // BLK)
            blk_gid[bpos:bpos + nb_g] = g - gs
            bpos += nb_g

        # interleave within slices
        perm = np.empty(nodes_pad, dtype=np.int64)
        off = 0
        while off < nodes_pad:
            ln = min(SL, nodes_pad - off)
            e = ln // BLK
            src = idx[off:off + ln].reshape(e, BLK)      # [block, elem]
            perm[off:off + ln] = src.T.reshape(-1)       # col i + k*e
            off += ln

        xsrc = np.concatenate([x[ns:ne], np.zeros((1, D), np.float32)])
        xt = np.ascontiguousarray(
            xsrc[perm].T.astype(np.float16))             # [-1] -> zero row

        m8 = np.zeros((128, nt * G_PAD), dtype=np.float16)
        for t in range(nt):
            gid = blk_gid[t * 128:(t + 1) * 128]
            ok = gid >= 0
            m8[np.nonzero(ok)[0], t * G_PAD + gid[ok]] = 1.0
        import ml_dtypes
        m8 = m8.astype(ml_dtypes.float8_e4m3fn)

        aux = np.zeros((128, G_PAD + nc2), dtype=np.float16)
        aux[:, 0:G_PAD] = np.arange(G_PAD, dtype=np.float16)[None, :]
        aux[:, G_PAD:] = rel.astype(np.float16)

        rows = np.zeros((1, 4 * 128 + 2 * G_PAD), dtype=np.float16)
        rows[0, 0:128] = b1[0:128]
        rows[0, 128:256] = b1[128:256]
        rows[0, 256:384] = b2[0:128]
        rows[0, 384:512] = b2[128:256]
        rows[0, 512 + ngl:512 + G_PAD] = 0.0
        rows[0, 512:512 + ngl] = counts[gs:ge].astype(np.float16)
        rows[0, 512 + G_PAD:512 + 2 * G_PAD] = 1.0

        gpj = np.zeros((128, nj), dtype=np.float32)
        for j in range(nj):
            gpj[:, j] = gp + j

        in_maps.append({
            "xt": xt,
            "m8": m8,
            "wk": wk,
            "aux": aux,
            "rows": rows,
            "gpj": gpj,
            "bcv": np.full((1, 1), bc[0], dtype=np.float32),
        })

    res = run_bass_kernel_spmd(nc, in_maps, core_ids=list(range(N_CORES)))
    outs = []
    for k in range(N_CORES):
        o = res.results[k]["out"].reshape(-1)
        outs.append(o[: node_cnt[k]])
    return np.concatenate(outs).reshape(n, 1).astype(np.float32)
